# revision 32
# baseline (speedup 1.0000x reference)
"""Round 6: host-packed weight+sample0 blob, strided matmul views, no copies.

Trainium2 Bass kernel for a 3x3 stride-1 pad-1 conv:
x (32,128,64,64) f32, weight (256,128,3,3) f32, bias (256,) f32
-> out (32,256,64,64) f32.

Data-parallel over batch across 8 NeuronCores (4 samples each); conv as
9 shifted fp16 matmuls per 8-row output tile accumulating in fp32 PSUM
(fp16 is the precision floor here: fp8/DoubleRow fails the 2e-2 gate
even for 2 of 9 taps - measured 2.1e-2 - and bass has no int8 matmul).

Horizontal padding comes for free: the host pads x to width 66 with zero
columns, so tap (kh, kw) is the strided SBUF view st[:, r0:r0+8, kw:kw+64]
(row stride 66) - no shifted copies, no memsets, no measurable PE cost.
Vertical padding via range-restricted matmuls on edge tiles (taps ordered
so the first matmul of each accumulation group covers the full PSUM range).

Head: weights and sample 0 are packed into one host-built blob tensor,
fetched as small need-ordered chunks on the sync DGE ring (a DMA's
completion semaphore trails its own serially-dispatched engines, so
several small chunks complete earlier than one big one). The first conv
tile is split into two 4-row strips so the first matmuls need only
x0 rows 0-4. Bulk sample loads are pushed behind the first output stores
via scheduling-time hints so they cannot steal DMA-engine bandwidth from
the head. 11 identity transposes keep the PE busy from the ordering
barrier until the head lands (~10.7us), which also starts the HAM clock
ramp (1.2->2.4 GHz after ~3.4us sustained activity) with no gap - a
sub-us PE gap before the flip resets the ramp window.

Output pairs (two 8-row tiles) are bias-added on the scalar engine and
stored fp16, alternating between both DGE rings; the final tile is split
4+2+2 rows so the closing matmul->drain->store->completion chain is short.
"""

import numpy as np

import concourse.bass as bass
from concourse import bacc
import concourse.mybir as mybir
import concourse.tile as tile
from concourse.bass_utils import run_bass_kernel_spmd
from concourse.masks import make_identity

N_CORES = 8
B_FULL = 32
B_LOCAL = B_FULL // N_CORES  # 4
CI = 128
CO = 256
H = W = 64
WP = W + 2  # host-padded width (zero col at 0 and 65)
ROWS = 8  # output rows per PSUM tile -> free dim 8*64 = 512
N_T = H // ROWS  # 8
F32 = mybir.dt.float32
F16 = mybir.dt.float16

# Blob layout (f16 elements per partition), in need-order:
#   [0    :  384) A1 = cb0 kh1 taps (kw-minor, 128 each)
#   [384  : 4608) x0 = sample-0 rows, 64*66
#   [4608 : 5376) A2 = cb0 kh2 + cb0 kh0 taps
#   [5376 : 6528) B  = cb1 kh0..kh2 taps
X0 = 384
NX = H * WP  # 4224
A2_OFF = X0 + NX  # 4608
B_OFF = A2_OFF + 768  # 5376
BLOB = B_OFF + 1152  # 6528
TAP = {(0, 1): 0, (0, 2): A2_OFF, (0, 0): A2_OFF + 384,
       (1, 0): B_OFF, (1, 1): B_OFF + 384, (1, 2): B_OFF + 768}

# Blob chunk schedule (element ranges), one DMA each on the sync ring:
# A1, x0 rows 0-8, A2, rows 9-16, rows 17-32, B, rows 33-63. Small
# leading chunks: a DMA's completion semaphore waits on its own last
# serially-dispatched engine, so pipelined small DMAs complete earlier
# than one merged chunk.
CHUNKS = [
    (0, X0),
    (X0, X0 + 5 * WP),
    (A2_OFF, B_OFF),
    (X0 + 5 * WP, X0 + 9 * WP),
    (X0 + 9 * WP, X0 + 17 * WP),
    (X0 + 17 * WP, X0 + 33 * WP),
    (B_OFF, BLOB),
    (X0 + 33 * WP, A2_OFF),
]


def build_nc():
    nc = bacc.Bacc()
    blob_d = nc.dram_tensor("blob", [CI, BLOB], F16, kind="ExternalInput")
    xr_d = nc.dram_tensor("xr", [B_LOCAL - 1, CI, H, WP], F16, kind="ExternalInput")
    b_d = nc.dram_tensor("bias", [CO], F32, kind="ExternalInput")
    o_d = nc.dram_tensor("out", [B_LOCAL, CO, H, W], F16, kind="ExternalOutput")

    with tile.TileContext(nc) as tc:
        with (
            tc.tile_pool(name="const", bufs=1) as const,
            tc.tile_pool(name="xstage", bufs=B_LOCAL - 1) as xstage,
            tc.tile_pool(name="obuf", bufs=5) as opool,
            tc.tile_pool(name="psum", bufs=6, space="PSUM") as pspool,
            tc.tile_pool(name="psum_tr", bufs=2, space="PSUM") as trpool,
        ):
            ident = const.tile([128, 128], F32)
            # First PE activity ASAP: two fp16 matmuls on a vector-zeroed
            # region run while gpsimd is still building the identity,
            # starting the HAM clock ramp ~0.3us earlier. The zero tile
            # comes from the obuf pool, which sits AFTER the blob in SBUF,
            # so the blob's banking-sensitive base address is unchanged.
            zwarm = opool.tile([128, 256], F16, name="zwarm", tag="zw")
            nc.vector.memset(zwarm, 0.0)
            for _ in range(3):
                warm = trpool.tile([128, 128], F32, tag="tr")
                nc.tensor.matmul(
                    warm, zwarm[:, 0:128], zwarm[:, 128:256], start=True, stop=True
                )
            make_identity(nc, ident)
            # PE busy from the ordering barrier until the head chunk lands
            # (~11.3us: DMA issue + ring spin-up + serial sub-descriptor
            # dispatch across the 16 DMA engines + completion semaphore).
            # 10 transposes end right there, with the HAM clock flip
            # (~3.4us after the first warm matmul) landing just before - so
            # conv matmuls start warm with no PE gap. NOTE: do not insert
            # SBUF allocations before `blob` - shifting its base address
            # costs ~42ns on every conv matmul (SBUF read-stream banking).
            for _ in range(10):
                warm = trpool.tile([128, 128], F32, tag="tr")
                nc.tensor.transpose(warm, ident, ident)

            blob = const.tile([128, BLOB], F16)
            bias_sb = const.tile([128, 2], F32)
            stages = [None] + [
                xstage.tile([128, H, WP], F16, name=f"st{b}", tag=f"st{b}")
                for b in range(1, B_LOCAL)
            ]

            for a, b_ in CHUNKS:
                nc.sync.dma_start(blob[:, a:b_], blob_d[:, a:b_])
            nc.scalar.dma_start(bias_sb, b_d.rearrange("(cb cp) -> cp cb", cb=2))
            # Bulk sample loads, placed behind the first output stores via
            # scheduling-time hints (emission order alone gets hoisted by
            # the tile scheduler). At runtime each lands behind a store
            # whose drain fires at ~15-25us - long before samples 1-3 are
            # consumed (~42/73/103us) - keeping the early DMA rings clear
            # for the critical head chunks.
            with tc.tile_wait_until(0.016):
                nc.sync.dma_start(stages[1], xr_d[0])
            with tc.tile_wait_until(0.018):
                nc.scalar.dma_start(stages[2], xr_d[1])
            with tc.tile_wait_until(0.025):
                nc.scalar.dma_start(stages[3], xr_d[2])

            xv0 = blob[:, X0:A2_OFF].rearrange("p (h w) -> p h w", w=WP)
            o_v = o_d.rearrange("b (cb cp) h w -> b cb cp (h w)", cb=2)

            def xview(b):
                return xv0 if b == 0 else stages[b]

            def kh_order(cb, t):
                # First tap of each group must cover the full PSUM range
                # (start=True clears the whole bank's has_written). kh1 is
                # always full for cb0 (incl. t=0/t=7); kh0 is full for t>0.
                if cb == 0 or t == 0:
                    return (1, 2, 0)
                return (0, 1, 2)

            def emit_taps(ps, st, cb, h0, n_out, kh_seq):
                """n_out output rows starting at h0, into ps[:, :n_out*W]."""
                i = 0
                n_mm = 3 * len(kh_seq)
                for kh in kh_seq:
                    r0 = h0 + kh - 1
                    rs, re = max(r0, 0), min(r0 + n_out, H)
                    a = (rs - r0) * W
                    b_ = a + (re - rs) * W
                    for kw in range(3):
                        nc.tensor.matmul(
                            ps[:, a:b_],
                            blob[:, TAP[(cb, kh)] + kw * 128 : TAP[(cb, kh)] + (kw + 1) * 128],
                            st[:, rs:re, kw : kw + W],
                            start=(i == 0),
                            stop=(i == n_mm - 1),
                        )
                        i += 1

            # Output tiles are drained (bias-add, fp16 cast) per PSUM tile
            # but stored one pair (t even, t odd) at a time: half the DMA
            # issues and completion semaphores.
            pair_obs = {}

            def conv_tile(b, cb, t):
                h0 = t * ROWS
                key = (b, cb, t // 2)
                if (b, cb, t) == (0, 0, 0):
                    # Split the very first tile into two 4-row strips so the
                    # first matmuls only need x0 rows 0-4 (a smaller, earlier
                    # head DMA).
                    ob0 = pair_obs[key] = opool.tile(
                        [128, 2 * ROWS * W], F16, name="ob", tag="ob"
                    )
                    for hh0 in (0, 4):
                        ps = pspool.tile([128, ROWS * W], F32)
                        emit_taps(ps[:, : 4 * W], xview(b), cb, hh0, 4, (1, 2, 0))
                        nc.scalar.add(
                            ob0[:, hh0 * W : (hh0 + 4) * W], ps[:, : 4 * W],
                            bias_sb[:, cb : cb + 1],
                        )
                    return
                ps = pspool.tile([128, ROWS * W], F32)
                emit_taps(ps, xview(b), cb, h0, ROWS, kh_order(cb, t))
                if key not in pair_obs:
                    pair_obs[key] = opool.tile(
                        [128, 2 * ROWS * W], F16, name="ob", tag="ob"
                    )
                ob = pair_obs[key]
                half = t % 2
                sl = slice(half * ROWS * W, (half + 1) * ROWS * W)
                nc.scalar.add(ob[:, sl], ps, bias_sb[:, cb : cb + 1])
                if half == 1:
                    # Alternate pair stores across both HWDGE rings: halves
                    # per-ring serialization and overlaps the final stores.
                    eng = nc.scalar if (b + cb + t // 2) % 2 else nc.sync
                    eng.dma_start(
                        o_v[b, cb, :, (t - 1) * ROWS * W : (t + 1) * ROWS * W], ob
                    )

            def penultimate_tile(b, cb, t):
                # Pair partner of the final tile: store alone so the final
                # tile can stream out in small strips.
                h0 = t * ROWS
                ps = pspool.tile([128, ROWS * W], F32, name="ps")
                emit_taps(ps, xview(b), cb, h0, ROWS, kh_order(cb, t))
                ob = opool.tile([128, ROWS * W], F16, name="obp", tag="obt")
                nc.scalar.add(ob, ps, bias_sb[:, cb : cb + 1])
                nc.sync.dma_start(o_v[b, cb, :, h0 * W : (h0 + ROWS) * W], ob)

            def final_tile(b, cb, t):
                # 4+2+2 rows: each strip's drain+store+completion hides
                # under the next strip's matmuls, shortening the
                # end-of-kernel chain.
                h0 = t * ROWS
                strips = [(h0, 4), (h0 + 4, 2), (h0 + 6, 2)]
                for si, (hh0, nr) in enumerate(strips):
                    ps = pspool.tile([128, ROWS * W], F32, name="ps")
                    emit_taps(ps[:, : nr * W], xview(b), cb, hh0, nr, (0, 1, 2))
                    ob = opool.tile([128, nr * W], F16, name="obq", tag="obt")
                    o_ap = o_v[b, cb, :, hh0 * W : (hh0 + nr) * W]
                    if si == 2:
                        nc.vector.tensor_scalar_add(
                            ob, ps[:, : nr * W], bias_sb[:, cb : cb + 1]
                        )
                        nc.sync.dma_start(o_ap, ob)
                    else:
                        nc.scalar.add(ob, ps[:, : nr * W], bias_sb[:, cb : cb + 1])
                        (nc.sync if si == 0 else nc.scalar).dma_start(o_ap, ob)

            n_total = 2 * N_T * B_LOCAL
            n_done = 0
            for b in range(B_LOCAL):
                for cb in range(2):
                    for t in range(N_T):
                        if n_done == n_total - 2:
                            penultimate_tile(b, cb, t)
                        elif n_done == n_total - 1:
                            final_tile(b, cb, t)
                        else:
                            conv_tile(b, cb, t)
                        n_done += 1

    nc.finalize()
    return nc


def _host_pack(x, weight):
    # x pad: [B, CI, H, W] f32 -> [B, CI, H, W+2] f16, zero edge cols.
    x_pad = np.zeros((B_FULL, CI, H, WP), dtype=np.float16)
    x_pad[:, :, :, 1 : W + 1] = x

    # weight repack: [co, ci, kh, kw] -> per-tap [ci, 128] blocks, kw-minor,
    # tap order [cb0 kh1 | cb0 kh2 | cb0 kh0 | cb1 kh0..kh2].
    w5 = weight.reshape(2, CO // 2, CI, 3, 3).transpose(0, 3, 4, 2, 1)
    # w5: [cb, kh, kw, ci, cp]
    a1 = w5[0][[1]].reshape(3, CI, 128)  # cb0 kh1, kw-minor
    a2 = w5[0][[2, 0]].reshape(6, CI, 128)  # cb0 kh2, kh0
    bb = w5[1].reshape(9, CI, 128)  # cb1 kh0..kh2
    def flat(wblk):  # [taps, ci, cp] -> [ci, taps*128]
        return wblk.transpose(1, 0, 2).reshape(CI, -1)
    return x_pad, flat(a1), flat(a2), flat(bb)


def run(x: np.ndarray, weight: np.ndarray, bias: np.ndarray, **spmd_kwargs):
    weight = np.ascontiguousarray(weight, dtype=np.float32)
    bias = np.ascontiguousarray(bias, dtype=np.float32)
    x_pad, a1, a2, bb = _host_pack(x, weight)

    nc = build_nc()
    in_maps = []
    for c in range(N_CORES):
        x0 = x_pad[c * B_LOCAL].reshape(CI, NX)
        blob = np.concatenate([a1, x0.astype(np.float16), a2, bb], axis=1)
        in_maps.append(
            {
                "blob": np.ascontiguousarray(blob).astype(np.float16),
                "xr": x_pad[c * B_LOCAL + 1 : (c + 1) * B_LOCAL],
                "bias": bias,
            }
        )
    res = run_bass_kernel_spmd(
        nc, in_maps, core_ids=list(range(N_CORES)), **spmd_kwargs
    )
    out = np.concatenate(
        [np.asarray(r["out"]).astype(np.float32) for r in res.results], axis=0
    )
    return out, res


def kernel(x: np.ndarray, weight: np.ndarray, bias: np.ndarray) -> np.ndarray:
    out, _ = run(x, weight, bias)
    return out


# revision 34
# speedup vs baseline: 1.0214x; 1.0214x over previous
"""Round 6: host-packed weight+sample0 blob, strided matmul views, no copies.

Trainium2 Bass kernel for a 3x3 stride-1 pad-1 conv:
x (32,128,64,64) f32, weight (256,128,3,3) f32, bias (256,) f32
-> out (32,256,64,64) f32.

Data-parallel over batch across 8 NeuronCores (4 samples each); conv as
9 shifted fp16 matmuls per 8-row output tile accumulating in fp32 PSUM
(fp16 is the precision floor here: fp8/DoubleRow fails the 2e-2 gate
even for 2 of 9 taps - measured 2.1e-2 - and bass has no int8 matmul).

Horizontal padding comes for free: the host pads x to width 66 with zero
columns, so tap (kh, kw) is the strided SBUF view st[:, r0:r0+8, kw:kw+64]
(row stride 66) - no shifted copies, no memsets, no measurable PE cost.
Vertical padding via range-restricted matmuls on edge tiles (taps ordered
so the first matmul of each accumulation group covers the full PSUM range).

Head: weights and sample 0 are packed into one host-built blob tensor,
fetched as small need-ordered chunks on the sync DGE ring (a DMA's
completion semaphore trails its own serially-dispatched engines, so
several small chunks complete earlier than one big one). The first conv
tile is split into two 4-row strips so the first matmuls need only
x0 rows 0-4. Bulk sample loads are pushed behind the first output stores
via scheduling-time hints so they cannot steal DMA-engine bandwidth from
the head. Three fp16 warm matmuls on a zeroed tile plus 10 identity
transposes keep the PE busy from the ordering barrier until the head
lands (~10.9us), which also starts the HAM clock ramp (1.2->2.4 GHz
after ~3.4us sustained activity) with no gap - a sub-us PE gap before
the flip resets the ramp window.

Output pairs (two 8-row tiles) are bias-added on the scalar engine and
stored fp16, alternating between both DGE rings; the final tile is split
4+2+2 rows so the closing matmul->drain->store->completion chain is short.
"""

import numpy as np

import concourse.bass as bass
from concourse import bacc
import concourse.mybir as mybir
import concourse.tile as tile
from concourse.bass_utils import run_bass_kernel_spmd
from concourse.masks import make_identity

N_CORES = 8
B_FULL = 32
B_LOCAL = B_FULL // N_CORES  # 4
CI = 128
CO = 256
H = W = 64
WP = W + 2  # host-padded width (zero col at 0 and 65)
ROWS = 8  # output rows per PSUM tile -> free dim 8*64 = 512
N_T = H // ROWS  # 8
F32 = mybir.dt.float32
F16 = mybir.dt.float16

# Blob layout (f16 elements per partition), in need-order:
#   [0    :  384) A1 = cb0 kh1 taps (kw-minor, 128 each)
#   [384  : 4608) x0 = sample-0 rows, 64*66
#   [4608 : 5376) A2 = cb0 kh2 + cb0 kh0 taps
#   [5376 : 6528) B  = cb1 kh0..kh2 taps
X0 = 384
NX = H * WP  # 4224
A2_OFF = X0 + NX  # 4608
B_OFF = A2_OFF + 768  # 5376
BLOB = B_OFF + 1152  # 6528
TAP = {(0, 1): 0, (0, 2): A2_OFF, (0, 0): A2_OFF + 384,
       (1, 0): B_OFF, (1, 1): B_OFF + 384, (1, 2): B_OFF + 768}

# Blob chunk schedule (element ranges), one DMA each on the sync ring:
# A1, x0 rows 0-8, A2, rows 9-16, rows 17-32, B, rows 33-63. Small
# leading chunks: a DMA's completion semaphore waits on its own last
# serially-dispatched engine, so pipelined small DMAs complete earlier
# than one merged chunk.
CHUNKS = [
    (0, X0),
    (X0, X0 + 5 * WP),
    (A2_OFF, B_OFF),
    (X0 + 5 * WP, X0 + 9 * WP),
    (X0 + 9 * WP, X0 + 17 * WP),
    (X0 + 17 * WP, X0 + 25 * WP),
    (X0 + 25 * WP, X0 + 33 * WP),
    (X0 + 33 * WP, A2_OFF),
    (B_OFF, BLOB),
]


def build_nc():
    nc = bacc.Bacc()
    blob_d = nc.dram_tensor("blob", [CI, BLOB], F16, kind="ExternalInput")
    xr_d = nc.dram_tensor("xr", [B_LOCAL - 1, CI, H, WP], F16, kind="ExternalInput")
    b_d = nc.dram_tensor("bias", [CO], F32, kind="ExternalInput")
    o_d = nc.dram_tensor("out", [B_LOCAL, CO, H, W], F16, kind="ExternalOutput")

    with tile.TileContext(nc) as tc:
        with (
            tc.tile_pool(name="const", bufs=1) as const,
            tc.tile_pool(name="xstage", bufs=B_LOCAL - 1) as xstage,
            tc.tile_pool(name="obuf", bufs=5) as opool,
            tc.tile_pool(name="psum", bufs=6, space="PSUM") as pspool,
            tc.tile_pool(name="psum_tr", bufs=2, space="PSUM") as trpool,
        ):
            ident = const.tile([128, 128], F32)
            # First PE activity ASAP: two fp16 matmuls on a vector-zeroed
            # region run while gpsimd is still building the identity,
            # starting the HAM clock ramp ~0.3us earlier. The zero tile
            # comes from the obuf pool, which sits AFTER the blob in SBUF,
            # so the blob's banking-sensitive base address is unchanged.
            zwarm = opool.tile([128, 256], F16, name="zwarm", tag="zw")
            nc.vector.memset(zwarm, 0.0)
            for _ in range(3):
                warm = trpool.tile([128, 128], F32, tag="tr")
                nc.tensor.matmul(
                    warm, zwarm[:, 0:128], zwarm[:, 128:256], start=True, stop=True
                )
            make_identity(nc, ident)
            # PE busy from the ordering barrier until the head chunk lands
            # (~11.3us: DMA issue + ring spin-up + serial sub-descriptor
            # dispatch across the 16 DMA engines + completion semaphore).
            # 10 transposes end right there, with the HAM clock flip
            # (~3.4us after the first warm matmul) landing just before - so
            # conv matmuls start warm with no PE gap. NOTE: do not insert
            # SBUF allocations before `blob` - shifting its base address
            # costs ~42ns on every conv matmul (SBUF read-stream banking).
            for _ in range(10):
                warm = trpool.tile([128, 128], F32, tag="tr")
                nc.tensor.transpose(warm, ident, ident)

            blob = const.tile([128, BLOB], F16)
            bias_sb = const.tile([128, 2], F32)
            stages = [None] + [
                xstage.tile([128, H, WP], F16, name=f"st{b}", tag=f"st{b}")
                for b in range(1, B_LOCAL)
            ]

            for a, b_ in CHUNKS:
                nc.sync.dma_start(blob[:, a:b_], blob_d[:, a:b_])
            nc.scalar.dma_start(bias_sb, b_d.rearrange("(cb cp) -> cp cb", cb=2))
            # Bulk sample loads, placed behind the first output stores via
            # scheduling-time hints (emission order alone gets hoisted by
            # the tile scheduler). At runtime each lands behind a store
            # whose drain fires at ~15-25us - long before samples 1-3 are
            # consumed (~42/73/103us) - keeping the early DMA rings clear
            # for the critical head chunks.
            with tc.tile_wait_until(0.016):
                nc.sync.dma_start(stages[1], xr_d[0])
            with tc.tile_wait_until(0.018):
                nc.scalar.dma_start(stages[2], xr_d[1])
            with tc.tile_wait_until(0.025):
                nc.scalar.dma_start(stages[3], xr_d[2])

            xv0 = blob[:, X0:A2_OFF].rearrange("p (h w) -> p h w", w=WP)
            o_v = o_d.rearrange("b (cb cp) h w -> b cb cp (h w)", cb=2)

            def xview(b):
                return xv0 if b == 0 else stages[b]

            def kh_order(cb, t):
                # First tap of each group must cover the full PSUM range
                # (start=True clears the whole bank's has_written). kh1 is
                # always full for cb0 (incl. t=0/t=7); kh0 is full for t>0.
                if cb == 0 or t == 0:
                    return (1, 2, 0)
                return (0, 1, 2)

            def emit_taps(ps, st, cb, h0, n_out, kh_seq):
                """n_out output rows starting at h0, into ps[:, :n_out*W]."""
                i = 0
                n_mm = 3 * len(kh_seq)
                for kh in kh_seq:
                    r0 = h0 + kh - 1
                    rs, re = max(r0, 0), min(r0 + n_out, H)
                    a = (rs - r0) * W
                    b_ = a + (re - rs) * W
                    for kw in range(3):
                        nc.tensor.matmul(
                            ps[:, a:b_],
                            blob[:, TAP[(cb, kh)] + kw * 128 : TAP[(cb, kh)] + (kw + 1) * 128],
                            st[:, rs:re, kw : kw + W],
                            start=(i == 0),
                            stop=(i == n_mm - 1),
                        )
                        i += 1

            # Output tiles are drained (bias-add, fp16 cast) per PSUM tile
            # but stored one pair (t even, t odd) at a time: half the DMA
            # issues and completion semaphores.
            pair_obs = {}

            def conv_tile(b, cb, t):
                h0 = t * ROWS
                key = (b, cb, t // 2)
                if (b, cb, t) == (0, 0, 0):
                    # Split the very first tile into two 4-row strips so the
                    # first matmuls only need x0 rows 0-4 (a smaller, earlier
                    # head DMA).
                    ob0 = pair_obs[key] = opool.tile(
                        [128, 2 * ROWS * W], F16, name="ob", tag="ob"
                    )
                    for hh0 in (0, 4):
                        ps = pspool.tile([128, ROWS * W], F32)
                        emit_taps(ps[:, : 4 * W], xview(b), cb, hh0, 4, (1, 2, 0))
                        nc.scalar.add(
                            ob0[:, hh0 * W : (hh0 + 4) * W], ps[:, : 4 * W],
                            bias_sb[:, cb : cb + 1],
                        )
                    return
                ps = pspool.tile([128, ROWS * W], F32)
                emit_taps(ps, xview(b), cb, h0, ROWS, kh_order(cb, t))
                if key not in pair_obs:
                    pair_obs[key] = opool.tile(
                        [128, 2 * ROWS * W], F16, name="ob", tag="ob"
                    )
                ob = pair_obs[key]
                half = t % 2
                sl = slice(half * ROWS * W, (half + 1) * ROWS * W)
                nc.scalar.add(ob[:, sl], ps, bias_sb[:, cb : cb + 1])
                if half == 1:
                    # Alternate pair stores across both HWDGE rings: halves
                    # per-ring serialization and overlaps the final stores.
                    eng = nc.scalar if (b + cb + t // 2) % 2 else nc.sync
                    eng.dma_start(
                        o_v[b, cb, :, (t - 1) * ROWS * W : (t + 1) * ROWS * W], ob
                    )

            def penultimate_tile(b, cb, t):
                # Pair partner of the final tile: store alone so the final
                # tile can stream out in small strips.
                h0 = t * ROWS
                ps = pspool.tile([128, ROWS * W], F32, name="ps")
                emit_taps(ps, xview(b), cb, h0, ROWS, kh_order(cb, t))
                ob = opool.tile([128, ROWS * W], F16, name="obp", tag="obt")
                nc.scalar.add(ob, ps, bias_sb[:, cb : cb + 1])
                nc.sync.dma_start(o_v[b, cb, :, h0 * W : (h0 + ROWS) * W], ob)

            def final_tile(b, cb, t):
                # 4+2+2 rows: each strip's drain+store+completion hides
                # under the next strip's matmuls, shortening the
                # end-of-kernel chain.
                h0 = t * ROWS
                strips = [(h0, 4), (h0 + 4, 2), (h0 + 6, 2)]
                for si, (hh0, nr) in enumerate(strips):
                    ps = pspool.tile([128, ROWS * W], F32, name="ps")
                    emit_taps(ps[:, : nr * W], xview(b), cb, hh0, nr, (0, 1, 2))
                    ob = opool.tile([128, nr * W], F16, name="obq", tag="obt")
                    o_ap = o_v[b, cb, :, hh0 * W : (hh0 + nr) * W]
                    if si == 2:
                        nc.vector.tensor_scalar_add(
                            ob, ps[:, : nr * W], bias_sb[:, cb : cb + 1]
                        )
                        nc.sync.dma_start(o_ap, ob)
                    else:
                        nc.scalar.add(ob, ps[:, : nr * W], bias_sb[:, cb : cb + 1])
                        (nc.sync if si == 0 else nc.scalar).dma_start(o_ap, ob)

            n_total = 2 * N_T * B_LOCAL
            n_done = 0
            for b in range(B_LOCAL):
                for cb in range(2):
                    for t in range(N_T):
                        if n_done == n_total - 2:
                            penultimate_tile(b, cb, t)
                        elif n_done == n_total - 1:
                            final_tile(b, cb, t)
                        else:
                            conv_tile(b, cb, t)
                        n_done += 1

    nc.finalize()
    return nc


def _host_pack(x, weight):
    # x pad: [B, CI, H, W] f32 -> [B, CI, H, W+2] f16, zero edge cols.
    x_pad = np.zeros((B_FULL, CI, H, WP), dtype=np.float16)
    x_pad[:, :, :, 1 : W + 1] = x

    # weight repack: [co, ci, kh, kw] -> per-tap [ci, 128] blocks, kw-minor,
    # tap order [cb0 kh1 | cb0 kh2 | cb0 kh0 | cb1 kh0..kh2].
    w5 = weight.reshape(2, CO // 2, CI, 3, 3).transpose(0, 3, 4, 2, 1)
    # w5: [cb, kh, kw, ci, cp]
    a1 = w5[0][[1]].reshape(3, CI, 128)  # cb0 kh1, kw-minor
    a2 = w5[0][[2, 0]].reshape(6, CI, 128)  # cb0 kh2, kh0
    bb = w5[1].reshape(9, CI, 128)  # cb1 kh0..kh2
    def flat(wblk):  # [taps, ci, cp] -> [ci, taps*128]
        return wblk.transpose(1, 0, 2).reshape(CI, -1)
    return x_pad, flat(a1), flat(a2), flat(bb)


def run(x: np.ndarray, weight: np.ndarray, bias: np.ndarray, **spmd_kwargs):
    weight = np.ascontiguousarray(weight, dtype=np.float32)
    bias = np.ascontiguousarray(bias, dtype=np.float32)
    x_pad, a1, a2, bb = _host_pack(x, weight)

    nc = build_nc()
    in_maps = []
    for c in range(N_CORES):
        x0 = x_pad[c * B_LOCAL].reshape(CI, NX)
        blob = np.concatenate([a1, x0.astype(np.float16), a2, bb], axis=1)
        in_maps.append(
            {
                "blob": np.ascontiguousarray(blob).astype(np.float16),
                "xr": x_pad[c * B_LOCAL + 1 : (c + 1) * B_LOCAL],
                "bias": bias,
            }
        )
    res = run_bass_kernel_spmd(
        nc, in_maps, core_ids=list(range(N_CORES)), **spmd_kwargs
    )
    out = np.concatenate(
        [np.asarray(r["out"]).astype(np.float32) for r in res.results], axis=0
    )
    return out, res


def kernel(x: np.ndarray, weight: np.ndarray, bias: np.ndarray) -> np.ndarray:
    out, _ = run(x, weight, bias)
    return out
